# revision 4
# baseline (speedup 1.0000x reference)
"""Trainium2 Bass kernel for nn_FactoredYiJingQuantizer.

Math: the 8 trigrams are all sign vectors {-1,+1}^3, so the softmax over
codebook entries factorizes per coordinate:
    w_k ∝ exp(-(|z|^2 - 2<z,s_k> + 3)/T) ∝ prod_d exp(2 z_d s_{k,d} / T)
    E[s_d] = tanh(2 z_d / T)
and the straight-through output x + sg(q - x) is numerically just q.
Hence the whole module is elementwise  y = tanh(x * 2/TEMP)  with
TEMP = 0.3 — a pure memory-bound elementwise kernel.

Sharding: data-parallel over the batch dim across 8 NeuronCores.
"""

import numpy as np

import concourse.bacc as bacc
import concourse.mybir as mybir
from concourse.bass_utils import run_bass_kernel_spmd
from concourse.tile import TileContext

N_CORES = 8
B, S, D = 2048, 8192, 6
ROWS_PER_CORE = B // N_CORES                 # 256
ELEMS_PER_CORE = ROWS_PER_CORE * S * D       # 12,582,912
P = 128                                      # SBUF partitions
TILE_F = 8192                                # free-dim elems per tile
N_TILES = ELEMS_PER_CORE // (P * TILE_F)     # 12
assert N_TILES * P * TILE_F == ELEMS_PER_CORE
TEMP = 0.3
SCALE = 2.0 / TEMP

_CACHE: dict = {}


def build_bass(tile_f: int = TILE_F, bufs: int = 4):
    n_tiles = ELEMS_PER_CORE // (P * tile_f)
    assert n_tiles * P * tile_f == ELEMS_PER_CORE
    nc = bacc.Bacc(num_devices=N_CORES)
    x = nc.declare_dram_parameter(
        "x", [n_tiles, P, tile_f], mybir.dt.float32, isOutput=False
    )
    y = nc.declare_dram_parameter(
        "y", [n_tiles, P, tile_f], mybir.dt.float32, isOutput=True
    )
    with TileContext(nc) as tc:
        with tc.tile_pool(name="io", bufs=bufs) as pool:
            for t in range(n_tiles):
                tile = pool.tile([P, tile_f], mybir.dt.float32)
                nc.sync.dma_start(out=tile[:], in_=x[t])
                nc.scalar.activation(
                    tile[:], tile[:], mybir.ActivationFunctionType.Tanh, scale=SCALE
                )
                nc.sync.dma_start(out=y[t], in_=tile[:])
    nc.compile()
    return nc


def shard_inputs(x: np.ndarray) -> list[dict[str, np.ndarray]]:
    tile_f = _CACHE.get("tile_f", TILE_F)
    n_tiles = ELEMS_PER_CORE // (P * tile_f)
    shards = np.ascontiguousarray(x, dtype=np.float32).reshape(
        N_CORES, n_tiles, P, tile_f
    )
    return [{"x": shards[i]} for i in range(N_CORES)]


def kernel(x: np.ndarray) -> np.ndarray:
    x = np.asarray(x)
    assert x.shape == (B, S, D), x.shape
    if "nc" not in _CACHE:
        _CACHE["tile_f"] = TILE_F
        _CACHE["nc"] = build_bass(TILE_F)
    nc = _CACHE["nc"]
    in_maps = shard_inputs(x)
    res = run_bass_kernel_spmd(nc, in_maps, list(range(N_CORES)))
    out = np.stack([res.results[i]["y"] for i in range(N_CORES)])
    return out.reshape(B, S, D).astype(np.float32, copy=False)
